# revision 34
# baseline (speedup 1.0000x reference)
"""Causal multi-head attention (B=8, L=1024, D_IN=512, H=8, D=64) on 8 TRN2
NeuronCores, data-parallel over batch (one batch element per core, no
collectives).

v3 design (per core, batch element b):
  host:   QsT/KsT/VsT = seq[b].T as bf16 [512, 1024]; weights bf16 [512, 512]
          (WQ pre-scaled by 1/sqrt(D) so the softmax scale is free).
  device: qT = WQ.T @ QsT -> [512(dout), 1024(L)] (heads on partitions: head
          2t on partitions 0-63, head 2t+1 on 64-127 of dout tile t); kT
          likewise; v = (VsT.T @ WV) stored [L, H, 66] with ones columns.
          Attention per head-pair t, per query chunk qc (512 cols):
            S^T pair matmuls ROW-TILED (K=64 -> tile_position (0,0)/(64,0))
            so both heads' score matmuls stream concurrently into separate
            PSUM banks of one [128, 2, 512] tile; one exp ACT covers both
            heads; causal mask = 0/1 multiply on the diagonal block; O^T
            accumulated over key tiles with lhsT = [v_h | 1 | 1].
          Pair/chunk groups are streamed in ORDER = [t0qc0, t1qc0, t0qc1,
          t1qc1, t2qc0, ...]: the PE engine queue is FIFO, so the schedule
          keeps DMA-ready work (first-L-half projections of pairs 0-1)
          ahead of work gated on the second sequence-half DMAs.  Projection
          chunks are interleaved between score matmuls (SLOTS) to fill the
          PE while ScalarE exp catches up.
  host:   OUT[h, :64, :] / OUT[h, 64, :] (bf16), transpose, concat heads.

A dependency-free chain of dummy matmuls runs first, overlapping the input
DMAs, so the PE's HAM clock gate opens (1.2 -> 2.4 GHz) before real work.
A tiny dummy exp preloads the ScalarE activation table set off the critical
path.  Each dma_start costs ~0.7-1us of HWDGE descriptor generation on its
issuing engine queue and all transfers share ~436 GB/s of SDMA bandwidth,
so the input staging is a single priority-ordered stream (wq, qsT-half1,
wk, ksT-half1, ...) rather than parallel rings: parallel rings round-robin
the bandwidth and delay the critical first tensors.
"""

import numpy as np
import ml_dtypes

B, L, D_IN = 8, 1024, 512
H, D = 8, 64
DA = D + 2  # head dim + two ones columns (denominator; padded even so the
# bf16 lhsT slices stay 4-byte aligned — odd column counts hang the HW)
N_CORES = 8
SCALE = 1.0 / np.sqrt(D).item()  # folded into WQ on the host
N_WARMUP = 7  # dummy matmuls to open the HAM clock gate during input DMA
N_WARM_MID = (2, 1)  # extra dummy matmuls after q0n0/k0n0 to bridge DMA waits

_GRAPH_CACHE = {}

# score-block groups per query chunk: list of jt lists; jts in one group
# share one spair PSUM tile + one exp ACT (packed along the free dim)
GROUPS_QC = (
    [[0], [1], [2, 3]],
    [[0], [1], [2], [3], [4], [5], [6, 7]],
)


def build_attention_body(tc, qsT, ksT, vsT, wq, wk, wv, mask, out):
    """Emit the per-core kernel into TileContext `tc` (APs per module doc)."""
    import contextlib

    import concourse.mybir as mybir

    nc = tc.nc
    fp32 = mybir.dt.float32
    bf16 = mybir.dt.bfloat16
    EXP = mybir.ActivationFunctionType.Exp

    with contextlib.ExitStack() as ctx:
        const = ctx.enter_context(tc.tile_pool(name="const", bufs=1))
        sb = ctx.enter_context(tc.tile_pool(name="sb", bufs=1))
        ppool = ctx.enter_context(tc.tile_pool(name="ppool", bufs=1))
        stage = ctx.enter_context(tc.tile_pool(name="stage", bufs=2))
        psum = ctx.enter_context(tc.tile_pool(name="psum", bufs=1, space="PSUM"))

        # ---- PE warm-up + ACT table preload, racing the input DMAs -------
        warm_sb = const.tile([128, 512], bf16)
        nc.gpsimd.memset(warm_sb[:], 0.0)
        warm_act = const.tile([128, 16], bf16)
        nc.scalar.activation(warm_act[:], warm_sb[:, 0:16], EXP)
        for i in range(N_WARMUP):
            pwarm = psum.tile(
                [128, 512], fp32, tag="proj", bufs=2, name=f"pwarm_{i}"
            )
            nc.tensor.matmul(
                pwarm[:], warm_sb[:, 0:128], warm_sb[:], start=True, stop=True
            )

        # ---- stage inputs into SBUF (ordered by first use) ---------------
        # One priority-ordered stream on the sync HWDGE ring: q/k first so
        # the score stream starts ASAP, sequence tensors split into L-halves
        # so nch=0 projection chunks start a half-DMA earlier, v last (its
        # projection is deferred into the attention stream).  All transfers
        # share one ~436 GB/s SDMA pool, so parallelizing across rings only
        # delays the critical first tensors; the tiny mask rides the scalar
        # ring since it is off the critical path.
        wq_sb = const.tile([128, 4, 512], bf16)
        nc.sync.dma_start(wq_sb[:], wq.rearrange("(kt p) n -> p kt n", p=128))
        qsT_sb = const.tile([128, 4, L], bf16)
        qsT_r = qsT.rearrange("(kt p) l -> p kt l", p=128)
        nc.sync.dma_start(qsT_sb[:, :, 0:512], qsT_r[:, :, 0:512])
        wk_sb = const.tile([128, 4, 512], bf16)
        nc.sync.dma_start(wk_sb[:], wk.rearrange("(kt p) n -> p kt n", p=128))
        ksT_sb = const.tile([128, 4, L], bf16)
        ksT_r = ksT.rearrange("(kt p) l -> p kt l", p=128)
        nc.sync.dma_start(ksT_sb[:, :, 0:512], ksT_r[:, :, 0:512])
        nc.sync.dma_start(qsT_sb[:, :, 512:L], qsT_r[:, :, 512:L])
        nc.sync.dma_start(ksT_sb[:, :, 512:L], ksT_r[:, :, 512:L])
        wv_sb = const.tile([128, 4, 512], bf16)
        nc.sync.dma_start(wv_sb[:], wv.rearrange("(kt p) n -> p kt n", p=128))
        vsT_sb = const.tile([128, 4, L], bf16)
        vsT_r = vsT.rearrange("(kt p) l -> p kt l", p=128)
        nc.sync.dma_start(vsT_sb[:, :, 0:512], vsT_r[:, :, 0:512])
        nc.sync.dma_start(vsT_sb[:, :, 512:L], vsT_r[:, :, 512:L])
        mask_sb = const.tile([128, 128], bf16)
        nc.scalar.dma_start(mask_sb[:], mask[:, :])

        # ---- persistent activations --------------------------------------
        qT_sb = sb.tile([128, 4, L], bf16)   # [dout%128, dout//128, L]
        kT_sb = sb.tile([128, 4, L], bf16)
        v_sb = sb.tile([128, 8, H, DA], bf16)  # [j%128, j//128, head, d|1|1]
        nc.vector.memset(v_sb[:, :, :, D:DA], 1.0)

        def proj_qk_chunk(t, which, nch):
            # one [128, 512] chunk of qT/kT tile t (lhsT = weight tile)
            dst, w_t, src = (
                (qT_sb, wq_sb, qsT_sb), (kT_sb, wk_sb, ksT_sb)
            )[which]
            pq = psum.tile(
                [128, 512], fp32, tag="proj", bufs=2,
                name=f"pq_{t}_{which}_{nch}",
            )
            for kt in range(4):
                nc.tensor.matmul(
                    pq[:],
                    w_t[:, kt, t * 128:(t + 1) * 128],
                    src[:, kt, nch * 512:(nch + 1) * 512],
                    start=(kt == 0),
                    stop=(kt == 3),
                )
            nc.vector.tensor_copy(
                out=dst[:, t, nch * 512:(nch + 1) * 512], in_=pq[:]
            )

        def proj_v(it):
            # v natural: v[i, n] = sum_k Vs[i, k] WV[k, n]; lhsT = VsT tile
            pv = psum.tile([128, 512], fp32, tag="proj", bufs=2, name=f"pv_{it}")
            for kt in range(4):
                nc.tensor.matmul(
                    pv[:],
                    vsT_sb[:, kt, it * 128:(it + 1) * 128],
                    wv_sb[:, kt, :],
                    start=(kt == 0),
                    stop=(kt == 3),
                )
            nc.vector.tensor_copy(
                out=v_sb[:, it, :, 0:D],
                in_=pv.rearrange("p (h d) -> p h d", h=H),
            )

        # ---- flat attention stream across all pairs ----------------------
        # Groups G0..G39 ordered so early groups only need the FIRST L-half
        # projections (t0qc0, t1qc0) while qsT2/ksT2 are still in flight;
        # the nch=1-gated groups (t0qc1) come after.  The PE engine queue is
        # FIFO, so a data-gated matmul at the queue head stalls everything
        # behind it — this order keeps DMA-ready work ahead of gated work.
        ORDER = [(0, 0), (1, 0), (0, 1), (1, 1), (2, 0), (2, 1), (3, 0), (3, 1)]
        flat = []  # (t, qc, group jts, last_of_qc)
        for t, qc in ORDER:
            gs = GROUPS_QC[qc]
            for gi, g in enumerate(gs):
                flat.append((t, qc, g, gi == len(gs) - 1))

        # slot proj chunks, one per step (smooth PE/Scalar interleave),
        # placed after their input DMA lands but before their consumer
        # group's scores: nch=1 chunks need the second sequence L-halves
        # (~8us), v chunks need vsT (~11-13us).
        SLOTS = {
            0: [("qk", 1, 0, 0)], 1: [("qk", 1, 1, 0)],
            3: [("qk", 0, 0, 1)], 4: [("qk", 0, 1, 1)],
            6: [("qk", 1, 0, 1)], 7: [("qk", 1, 1, 1)],
            8: [("qk", 2, 0, 0)], 10: [("qk", 2, 1, 0)],
            9: [("v", 0)], 11: [("v", 1)], 12: [("v", 2)], 13: [("v", 3)],
            14: [("v", 4)], 15: [("v", 5)], 16: [("v", 6)], 17: [("v", 7)],
            18: [("qk", 2, 0, 1)], 20: [("qk", 2, 1, 1)],
            22: [("qk", 3, 0, 0)], 24: [("qk", 3, 1, 0)],
            27: [("qk", 3, 0, 1)], 29: [("qk", 3, 1, 1)],
        }
        CAP1_UNTIL = 17  # O-drain cap is 1/step through the v window

        oT_d = {}  # (t, qc) -> oT psum tile, allocated at first O drain

        def drain_o(rec):
            t, qc, group, last_of_qc, colofs_l, pexp = rec
            qlo = 512 * qc
            nj = 4 * (qc + 1)
            if (t, qc) not in oT_d:
                oT_d[(t, qc)] = psum.tile(
                    [DA, 2, 512], fp32, tag="oT", bufs=1, name=f"oT_{t}_{qc}"
                )
            oT = oT_d[(t, qc)]
            for jt, cofs in zip(group, colofs_l):
                j0 = 128 * jt
                off = max(0, j0 - qlo)
                cw = 512 - off
                for hh in range(2):
                    nc.tensor.matmul(
                        oT[:, hh, off:off + cw],
                        v_sb[:, jt, 2 * t + hh, :],
                        pexp[:, hh, cofs:cofs + cw],
                        start=(jt == 0),
                        stop=(jt == nj - 1),
                        skip_group_check=True,
                    )
            if (t, qc) == (3, 1) and 5 in group:
                # final qc: cols [0:256] are final after jt5 — ship them while
                # the last two groups still run, so the kernel tail is short
                o_sta = stage.tile(
                    [DA, 2, 512], bf16, tag="ost", name="ost_3_1a"
                )
                nc.vector.tensor_copy(out=o_sta[:, :, 0:256], in_=oT[:, :, 0:256])
                for hh, eng in ((0, nc.sync), (1, nc.scalar)):
                    eng.dma_start(
                        out[2 * t + hh, :, qlo:qlo + 256], o_sta[:, hh, 0:256]
                    )
            if last_of_qc:
                o_st = stage.tile(
                    [DA, 2, 512], bf16, tag="ost", name=f"ost_{t}_{qc}"
                )
                lo = 256 if (t, qc) == (3, 1) else 0
                if lo == 0:
                    nc.vector.tensor_copy(out=o_st[:, :, 0:384],
                                          in_=oT[:, :, 0:384])
                    nc.vector.tensor_copy(out=o_st[:, :, 384:512],
                                          in_=oT[:, :, 384:512])
                else:
                    nc.vector.tensor_copy(out=o_st[:, :, 256:512],
                                          in_=oT[:, :, 256:512])
                # split the two head-DMAs across the two HWDGE rings so their
                # ~1us descriptor generations run concurrently
                for hh, eng in ((0, nc.sync), (1, nc.scalar)):
                    eng.dma_start(
                        out[2 * t + hh, :, qlo + lo:qlo + 512],
                        o_st[:, hh, lo:512],
                    )

        # lead-in: first projection chunks for pair 0 (first L-half only),
        # with dummy-matmul filler so the PE never idles past a HAM window
        # while the k/q DMAs land
        def warm_fill(n, base):
            for i in range(n):
                pw = psum.tile(
                    [128, 512], fp32, tag="proj", bufs=2,
                    name=f"pwm_{base}_{i}",
                )
                nc.tensor.matmul(
                    pw[:], warm_sb[:, 0:128], warm_sb[:],
                    start=True, stop=True,
                )

        proj_qk_chunk(0, 0, 0)
        warm_fill(N_WARM_MID[0], "a")
        proj_qk_chunk(0, 1, 0)
        warm_fill(N_WARM_MID[1], "b")

        pending = []   # group records awaiting O emission
        n_v = 0        # v chunks emitted so far (in it order 0..7)

        def emit_group(Gi):
            # S-pair matmuls -> exp ACT -> diag masks for flat group Gi
            t, qc, group, last_of_qc = flat[Gi]
            qlo = 512 * qc
            sp = psum.tile(
                [128, 2, 512], fp32, tag="spair", bufs=2, name=f"sp_{Gi}"
            )
            pexp = ppool.tile(
                [128, 2, 512], bf16, tag="P", bufs=14, name=f"P_{Gi}"
            )
            # first groups: high priority so the scheduler doesn't stuff
            # hoisted projection filler between the k0n0 cast and S(G0)
            ctx2 = None
            if Gi < 7:
                ctx2 = tc.high_priority(offset=50000)
                ctx2.__enter__()
            colofs_l = []
            colofs = 0
            for jt in group:
                j0 = 128 * jt
                off = max(0, j0 - qlo)
                cw = 512 - off
                for hh in range(2):
                    pb = 64 * hh
                    nc.tensor.matmul(
                        sp[:, hh, colofs:colofs + cw],
                        kT_sb[pb:pb + 64, t, j0:j0 + 128],
                        qT_sb[pb:pb + 64, t, qlo + off:qlo + 512],
                        start=True,
                        stop=True,
                    )
                colofs_l.append(colofs)
                colofs += cw
            nc.scalar.activation(pexp[:, :, 0:colofs], sp[:, :, 0:colofs], EXP)
            for jt, cofs in zip(group, colofs_l):
                if 128 * jt >= qlo:  # chunk starts at the diagonal
                    for hh in range(2):
                        nc.vector.tensor_mul(
                            pexp[:, hh, cofs:cofs + 128],
                            pexp[:, hh, cofs:cofs + 128],
                            mask_sb[:],
                        )
            if ctx2 is not None:
                ctx2.__exit__(None, None, None)
            pending.append((t, qc, group, last_of_qc, colofs_l, pexp))

        # Groups are emitted in PAIRS: S(G) and S(G+1) go back-to-back into
        # the two spair buffers, so ScalarE always has a 2-deep exp input
        # queue and rides out the projection/drain fill phases without
        # starving (exp consumes a group ~3x slower than the PE produces
        # one, but fill bursts between single S-pairs exceeded one exp).
        for base in range(0, len(flat), 2):
            emit_group(base)
            emit_group(base + 1)
            for Gi in (base, base + 1):
                for c in SLOTS.get(Gi, []):
                    if c[0] == "qk":
                        proj_qk_chunk(c[1], c[2], c[3])
                    else:
                        proj_v(c[1])
                        n_v += 1
            # drain eligible pending O groups: light through the v window
            # (PE-dense region), then work the backlog off
            cap = 2 if base + 1 <= 33 else 4
            drained = 0
            while pending and drained < cap:
                need_v = max(pending[0][2])
                if len(pending) > 1 and need_v < n_v:
                    drain_o(pending.pop(0))
                    drained += 1
                else:
                    break
        while pending:
            drain_o(pending.pop(0))


def _build_graph():
    import concourse.mybir as mybir
    import concourse.tile as tile
    from concourse import bacc

    nc = bacc.Bacc("TRN2", target_bir_lowering=False)
    bf16 = mybir.dt.bfloat16
    qsT = nc.dram_tensor("QsT", (D_IN, L), bf16, kind="ExternalInput")
    ksT = nc.dram_tensor("KsT", (D_IN, L), bf16, kind="ExternalInput")
    vsT = nc.dram_tensor("VsT", (D_IN, L), bf16, kind="ExternalInput")
    wq = nc.dram_tensor("WQ", (D_IN, H * D), bf16, kind="ExternalInput")
    wk = nc.dram_tensor("WK", (D_IN, H * D), bf16, kind="ExternalInput")
    wv = nc.dram_tensor("WV", (D_IN, H * D), bf16, kind="ExternalInput")
    mask = nc.dram_tensor("MASK", (128, 128), bf16, kind="ExternalInput")
    out = nc.dram_tensor("OUT", (H, DA, L), bf16, kind="ExternalOutput")

    with tile.TileContext(nc) as tc:
        build_attention_body(
            tc, qsT[:], ksT[:], vsT[:], wq[:], wk[:], wv[:], mask[:], out[:]
        )
    nc.compile()
    return nc


def get_graph():
    if "nc" not in _GRAPH_CACHE:
        _GRAPH_CACHE["nc"] = _build_graph()
    return _GRAPH_CACHE["nc"]


def make_in_maps(Q_seq, K_seq, V_seq, WQ, WK, WV):
    bf = ml_dtypes.bfloat16
    # fold the softmax 1/sqrt(D) into WQ so no scale is needed on-device
    wq = (np.asarray(WQ, dtype=np.float32) * SCALE).astype(bf)
    wk = np.asarray(WK, dtype=np.float32).astype(bf)
    wv = np.asarray(WV, dtype=np.float32).astype(bf)
    # keep-mask in S^T block coords: row r = key offset, col c = query offset;
    # keep key <= query  <=>  r <= c  (upper triangular incl. diagonal)
    mask = np.triu(np.ones((128, 128), dtype=np.float32)).astype(bf)
    in_maps = []
    for b in range(N_CORES):
        in_maps.append({
            "QsT": np.ascontiguousarray(np.asarray(Q_seq[b], np.float32).T).astype(bf),
            "KsT": np.ascontiguousarray(np.asarray(K_seq[b], np.float32).T).astype(bf),
            "VsT": np.ascontiguousarray(np.asarray(V_seq[b], np.float32).T).astype(bf),
            "WQ": wq,
            "WK": wk,
            "WV": wv,
            "MASK": mask,
        })
    return in_maps


def unshard(results):
    """results: list of per-core {"OUT": [H, DA, L] bf16} -> [B, L, H*D] f32."""
    outs = np.stack(
        [np.asarray(r["OUT"], np.float32) for r in results]
    )                                                    # [B, H, DA, L]
    o = outs[:, :, :D, :] / outs[:, :, D:D + 1, :]       # [B, H, D, L]
    return np.ascontiguousarray(
        o.transpose(0, 3, 1, 2).reshape(B, L, H * D)
    ).astype(np.float32)


def run(inputs, **run_kwargs):
    """Compile + run on the 8 cores; returns (output, BassKernelResults)."""
    from concourse.bass_utils import run_bass_kernel_spmd

    nc = get_graph()
    in_maps = make_in_maps(
        inputs["Q_seq"], inputs["K_seq"], inputs["V_seq"],
        inputs["WQ"], inputs["WK"], inputs["WV"],
    )
    res = run_bass_kernel_spmd(
        nc, in_maps, core_ids=list(range(N_CORES)), **run_kwargs
    )
    return unshard(res.results), res


def kernel(Q_seq, K_seq, V_seq, WQ, WK, WV):
    out, _ = run({
        "Q_seq": Q_seq, "K_seq": K_seq, "V_seq": V_seq,
        "WQ": WQ, "WK": WK, "WV": WV,
    })
    return out



# revision 35
# speedup vs baseline: 1.0399x; 1.0399x over previous
"""Causal multi-head attention (B=8, L=1024, D_IN=512, H=8, D=64) on 8 TRN2
NeuronCores, data-parallel over batch (one batch element per core, no
collectives).

v3 design (per core, batch element b):
  host:   QsT/KsT/VsT = seq[b].T as bf16 [512, 1024]; weights bf16 [512, 512]
          (WQ pre-scaled by 1/sqrt(D) so the softmax scale is free).
  device: qT = WQ.T @ QsT -> [512(dout), 1024(L)] (heads on partitions: head
          2t on partitions 0-63, head 2t+1 on 64-127 of dout tile t); kT
          likewise; v = (VsT.T @ WV) stored [L, H, 66] with ones columns.
          Attention per head-pair t, per query chunk qc (512 cols):
            S^T pair matmuls ROW-TILED (K=64 -> tile_position (0,0)/(64,0))
            so both heads' score matmuls stream concurrently into separate
            PSUM banks of one [128, 2, 512] tile; one exp ACT covers both
            heads; causal mask = 0/1 multiply on the diagonal block; O^T
            accumulated over key tiles with lhsT = [v_h | 1 | 1].
          Pair/chunk groups are streamed in ORDER = [t0qc0, t1qc0, t0qc1,
          t1qc1, t2qc0, ...]: the PE engine queue is FIFO, so the schedule
          keeps DMA-ready work (first-L-half projections of pairs 0-1)
          ahead of work gated on the second sequence-half DMAs.  Projection
          chunks are interleaved between score matmuls (SLOTS) to fill the
          PE while ScalarE exp catches up.
  host:   OUT[h, :64, :] / OUT[h, 64, :] (bf16), transpose, concat heads.

A dependency-free chain of dummy matmuls runs first, overlapping the input
DMAs, so the PE's HAM clock gate opens (1.2 -> 2.4 GHz) before real work.
A tiny dummy exp preloads the ScalarE activation table set off the critical
path.  Each dma_start costs ~0.7-1us of HWDGE descriptor generation on its
issuing engine queue and all transfers share ~436 GB/s of SDMA bandwidth,
so the input staging is a single priority-ordered stream (wq, qsT-half1,
wk, ksT-half1, ...) rather than parallel rings: parallel rings round-robin
the bandwidth and delay the critical first tensors.
"""

import numpy as np
import ml_dtypes

B, L, D_IN = 8, 1024, 512
H, D = 8, 64
DA = D + 2  # head dim + two ones columns (denominator; padded even so the
# bf16 lhsT slices stay 4-byte aligned — odd column counts hang the HW)
N_CORES = 8
SCALE = 1.0 / np.sqrt(D).item()  # folded into WQ on the host
N_WARMUP = 7  # dummy matmuls to open the HAM clock gate during input DMA
N_WARM_MID = (2, 1)  # extra dummy matmuls after q0n0/k0n0 to bridge DMA waits

_GRAPH_CACHE = {}

# score-block groups per query chunk: list of jt lists; jts in one group
# share one spair PSUM tile + one exp ACT (packed along the free dim)
GROUPS_QC = (
    [[0], [1], [2, 3]],
    [[0], [1], [2], [3], [4], [5], [6, 7]],
)


def build_attention_body(tc, qsT, ksT, vsT, wq, wk, wv, mask, out):
    """Emit the per-core kernel into TileContext `tc` (APs per module doc)."""
    import contextlib

    import concourse.mybir as mybir

    nc = tc.nc
    fp32 = mybir.dt.float32
    bf16 = mybir.dt.bfloat16
    EXP = mybir.ActivationFunctionType.Exp

    with contextlib.ExitStack() as ctx:
        const = ctx.enter_context(tc.tile_pool(name="const", bufs=1))
        sb = ctx.enter_context(tc.tile_pool(name="sb", bufs=1))
        ppool = ctx.enter_context(tc.tile_pool(name="ppool", bufs=1))
        stage = ctx.enter_context(tc.tile_pool(name="stage", bufs=2))
        psum = ctx.enter_context(tc.tile_pool(name="psum", bufs=1, space="PSUM"))

        # ---- PE warm-up + ACT table preload, racing the input DMAs -------
        warm_sb = const.tile([128, 512], bf16)
        nc.gpsimd.memset(warm_sb[:], 0.0)
        warm_act = const.tile([128, 16], bf16)
        nc.scalar.activation(warm_act[:], warm_sb[:, 0:16], EXP)
        for i in range(N_WARMUP):
            pwarm = psum.tile(
                [128, 512], fp32, tag="proj", bufs=2, name=f"pwarm_{i}"
            )
            nc.tensor.matmul(
                pwarm[:], warm_sb[:, 0:128], warm_sb[:], start=True, stop=True
            )

        # ---- stage inputs into SBUF (ordered by first use) ---------------
        # One priority-ordered stream on the sync HWDGE ring: q/k first so
        # the score stream starts ASAP, sequence tensors split into L-halves
        # so nch=0 projection chunks start a half-DMA earlier, v last (its
        # projection is deferred into the attention stream).  All transfers
        # share one ~436 GB/s SDMA pool, so parallelizing across rings only
        # delays the critical first tensors; the tiny mask rides the scalar
        # ring since it is off the critical path.
        wq_sb = const.tile([128, 4, 512], bf16)
        nc.sync.dma_start(wq_sb[:], wq.rearrange("(kt p) n -> p kt n", p=128))
        qsT_sb = const.tile([128, 4, L], bf16)
        qsT_r = qsT.rearrange("(kt p) l -> p kt l", p=128)
        nc.sync.dma_start(qsT_sb[:, :, 0:512], qsT_r[:, :, 0:512])
        wk_sb = const.tile([128, 4, 512], bf16)
        nc.sync.dma_start(wk_sb[:], wk.rearrange("(kt p) n -> p kt n", p=128))
        ksT_sb = const.tile([128, 4, L], bf16)
        ksT_r = ksT.rearrange("(kt p) l -> p kt l", p=128)
        nc.sync.dma_start(ksT_sb[:, :, 0:512], ksT_r[:, :, 0:512])
        nc.sync.dma_start(qsT_sb[:, :, 512:L], qsT_r[:, :, 512:L])
        nc.sync.dma_start(ksT_sb[:, :, 512:L], ksT_r[:, :, 512:L])
        wv_sb = const.tile([128, 4, 512], bf16)
        nc.sync.dma_start(wv_sb[:], wv.rearrange("(kt p) n -> p kt n", p=128))
        vsT_sb = const.tile([128, 4, L], bf16)
        vsT_r = vsT.rearrange("(kt p) l -> p kt l", p=128)
        nc.sync.dma_start(vsT_sb[:, :, 0:512], vsT_r[:, :, 0:512])
        nc.sync.dma_start(vsT_sb[:, :, 512:L], vsT_r[:, :, 512:L])
        mask_sb = const.tile([128, 128], bf16)
        nc.scalar.dma_start(mask_sb[:], mask[:, :])

        # ---- persistent activations --------------------------------------
        qT_sb = sb.tile([128, 4, L], bf16)   # [dout%128, dout//128, L]
        kT_sb = sb.tile([128, 4, L], bf16)
        v_sb = sb.tile([128, 8, H, DA], bf16)  # [j%128, j//128, head, d|1|1]
        nc.vector.memset(v_sb[:, :, :, D:DA], 1.0)

        def proj_qk_chunk(t, which, nch):
            # one [128, 512] chunk of qT/kT tile t (lhsT = weight tile)
            dst, w_t, src = (
                (qT_sb, wq_sb, qsT_sb), (kT_sb, wk_sb, ksT_sb)
            )[which]
            pq = psum.tile(
                [128, 512], fp32, tag="proj", bufs=2,
                name=f"pq_{t}_{which}_{nch}",
            )
            for kt in range(4):
                nc.tensor.matmul(
                    pq[:],
                    w_t[:, kt, t * 128:(t + 1) * 128],
                    src[:, kt, nch * 512:(nch + 1) * 512],
                    start=(kt == 0),
                    stop=(kt == 3),
                )
            nc.vector.tensor_copy(
                out=dst[:, t, nch * 512:(nch + 1) * 512], in_=pq[:]
            )

        def proj_v(it):
            # v natural: v[i, n] = sum_k Vs[i, k] WV[k, n]; lhsT = VsT tile
            pv = psum.tile([128, 512], fp32, tag="proj", bufs=2, name=f"pv_{it}")
            for kt in range(4):
                nc.tensor.matmul(
                    pv[:],
                    vsT_sb[:, kt, it * 128:(it + 1) * 128],
                    wv_sb[:, kt, :],
                    start=(kt == 0),
                    stop=(kt == 3),
                )
            nc.vector.tensor_copy(
                out=v_sb[:, it, :, 0:D],
                in_=pv.rearrange("p (h d) -> p h d", h=H),
            )

        # ---- flat attention stream across all pairs ----------------------
        # Groups G0..G39 ordered so early groups only need the FIRST L-half
        # projections (t0qc0, t1qc0) while qsT2/ksT2 are still in flight;
        # the nch=1-gated groups (t0qc1) come after.  The PE engine queue is
        # FIFO, so a data-gated matmul at the queue head stalls everything
        # behind it — this order keeps DMA-ready work ahead of gated work.
        ORDER = [(0, 0), (1, 0), (0, 1), (1, 1), (2, 0), (2, 1), (3, 0), (3, 1)]
        flat = []  # (t, qc, group jts, last_of_qc)
        for t, qc in ORDER:
            gs = GROUPS_QC[qc]
            for gi, g in enumerate(gs):
                flat.append((t, qc, g, gi == len(gs) - 1))

        # slot proj chunks, one per step (smooth PE/Scalar interleave),
        # placed after their input DMA lands but before their consumer
        # group's scores: nch=1 chunks need the second sequence L-halves
        # (~8us), v chunks need vsT (~11-13us).
        SLOTS = {
            0: [("qk", 1, 0, 0)], 1: [("qk", 1, 1, 0)],
            3: [("qk", 0, 0, 1)], 4: [("qk", 0, 1, 1)],
            6: [("qk", 1, 0, 1)], 7: [("qk", 1, 1, 1)],
            8: [("qk", 2, 0, 0)], 10: [("qk", 2, 1, 0)],
            9: [("v", 0)], 11: [("v", 1)], 12: [("v", 2)], 13: [("v", 3)],
            14: [("v", 4)], 15: [("v", 5)], 16: [("v", 6)], 17: [("v", 7)],
            18: [("qk", 2, 0, 1)], 20: [("qk", 2, 1, 1)],
            22: [("qk", 3, 0, 0)], 24: [("qk", 3, 1, 0)],
            27: [("qk", 3, 0, 1)], 29: [("qk", 3, 1, 1)],
        }
        CAP1_UNTIL = 17  # O-drain cap is 1/step through the v window

        oT_d = {}  # (t, qc) -> oT psum tile, allocated at first O drain

        def drain_o(rec):
            t, qc, group, last_of_qc, colofs_l, pexp = rec
            qlo = 512 * qc
            nj = 4 * (qc + 1)
            if (t, qc) not in oT_d:
                oT_d[(t, qc)] = psum.tile(
                    [DA, 2, 512], fp32, tag="oT", bufs=1, name=f"oT_{t}_{qc}"
                )
            oT = oT_d[(t, qc)]
            for jt, cofs in zip(group, colofs_l):
                j0 = 128 * jt
                off = max(0, j0 - qlo)
                cw = 512 - off
                for hh in range(2):
                    nc.tensor.matmul(
                        oT[:, hh, off:off + cw],
                        v_sb[:, jt, 2 * t + hh, :],
                        pexp[:, hh, cofs:cofs + cw],
                        start=(jt == 0),
                        stop=(jt == nj - 1),
                        skip_group_check=True,
                    )
            if (t, qc) == (3, 1) and 5 in group:
                # final qc: cols [0:256] are final after jt5 — ship them while
                # the last two groups still run, so the kernel tail is short
                o_sta = stage.tile(
                    [DA, 2, 512], bf16, tag="ost", name="ost_3_1a"
                )
                nc.vector.tensor_copy(out=o_sta[:, :, 0:256], in_=oT[:, :, 0:256])
                for hh, eng in ((0, nc.sync), (1, nc.scalar)):
                    eng.dma_start(
                        out[2 * t + hh, :, qlo:qlo + 256], o_sta[:, hh, 0:256]
                    )
            if last_of_qc:
                o_st = stage.tile(
                    [DA, 2, 512], bf16, tag="ost", name=f"ost_{t}_{qc}"
                )
                lo = 256 if (t, qc) == (3, 1) else 0
                if lo == 0:
                    nc.vector.tensor_copy(out=o_st[:, :, 0:384],
                                          in_=oT[:, :, 0:384])
                    nc.vector.tensor_copy(out=o_st[:, :, 384:512],
                                          in_=oT[:, :, 384:512])
                else:
                    nc.vector.tensor_copy(out=o_st[:, :, 256:512],
                                          in_=oT[:, :, 256:512])
                # split the two head-DMAs across the two HWDGE rings so their
                # ~1us descriptor generations run concurrently
                for hh, eng in ((0, nc.sync), (1, nc.scalar)):
                    eng.dma_start(
                        out[2 * t + hh, :, qlo + lo:qlo + 512],
                        o_st[:, hh, lo:512],
                    )

        # lead-in: first projection chunks for pair 0 (first L-half only),
        # with dummy-matmul filler so the PE never idles past a HAM window
        # while the k/q DMAs land
        def warm_fill(n, base):
            for i in range(n):
                pw = psum.tile(
                    [128, 512], fp32, tag="proj", bufs=2,
                    name=f"pwm_{base}_{i}",
                )
                nc.tensor.matmul(
                    pw[:], warm_sb[:, 0:128], warm_sb[:],
                    start=True, stop=True,
                )

        proj_qk_chunk(0, 0, 0)
        warm_fill(N_WARM_MID[0], "a")
        proj_qk_chunk(0, 1, 0)
        warm_fill(N_WARM_MID[1], "b")

        pending = []   # group records awaiting O emission
        n_v = 0        # v chunks emitted so far (in it order 0..7)

        def emit_group(Gi):
            # S-pair matmuls -> exp ACT -> diag masks for flat group Gi
            t, qc, group, last_of_qc = flat[Gi]
            qlo = 512 * qc
            sp = psum.tile(
                [128, 2, 512], fp32, tag="spair", bufs=2, name=f"sp_{Gi}"
            )
            pexp = ppool.tile(
                [128, 2, 512], bf16, tag="P", bufs=14, name=f"P_{Gi}"
            )
            # first groups: high priority so the scheduler doesn't stuff
            # hoisted projection filler between the k0n0 cast and S(G0)
            ctx2 = None
            if Gi < 7:
                ctx2 = tc.high_priority(offset=50000)
                ctx2.__enter__()
            colofs_l = []
            colofs = 0
            for jt in group:
                j0 = 128 * jt
                off = max(0, j0 - qlo)
                cw = 512 - off
                for hh in range(2):
                    pb = 64 * hh
                    nc.tensor.matmul(
                        sp[:, hh, colofs:colofs + cw],
                        kT_sb[pb:pb + 64, t, j0:j0 + 128],
                        qT_sb[pb:pb + 64, t, qlo + off:qlo + 512],
                        start=True,
                        stop=True,
                    )
                colofs_l.append(colofs)
                colofs += cw
            nc.scalar.activation(pexp[:, :, 0:colofs], sp[:, :, 0:colofs], EXP)
            for jt, cofs in zip(group, colofs_l):
                if 128 * jt >= qlo:  # chunk starts at the diagonal
                    for hh in range(2):
                        nc.vector.tensor_mul(
                            pexp[:, hh, cofs:cofs + 128],
                            pexp[:, hh, cofs:cofs + 128],
                            mask_sb[:],
                        )
            if ctx2 is not None:
                ctx2.__exit__(None, None, None)
            pending.append((t, qc, group, last_of_qc, colofs_l, pexp))

        # Groups are emitted in PAIRS: S(G) and S(G+1) go back-to-back into
        # the two spair buffers, so ScalarE always has a 2-deep exp input
        # queue and rides out the projection/drain fill phases without
        # starving (exp consumes a group ~3x slower than the PE produces
        # one, but fill bursts between single S-pairs exceeded one exp).
        for base in range(0, len(flat), 2):
            emit_group(base)
            emit_group(base + 1)
            for Gi in (base, base + 1):
                for c in SLOTS.get(Gi, []):
                    if c[0] == "qk":
                        proj_qk_chunk(c[1], c[2], c[3])
                    else:
                        proj_v(c[1])
                        n_v += 1
            # drain eligible pending O groups: light through the v window
            # (PE-dense region), then work the backlog off
            cap = 2 if base + 1 <= CAP1_UNTIL else (3 if base + 1 <= 33 else 4)
            drained = 0
            while pending and drained < cap:
                need_v = max(pending[0][2])
                if len(pending) > 1 and need_v < n_v:
                    drain_o(pending.pop(0))
                    drained += 1
                else:
                    break
        while pending:
            drain_o(pending.pop(0))


def _build_graph():
    import concourse.mybir as mybir
    import concourse.tile as tile
    from concourse import bacc

    nc = bacc.Bacc("TRN2", target_bir_lowering=False)
    bf16 = mybir.dt.bfloat16
    qsT = nc.dram_tensor("QsT", (D_IN, L), bf16, kind="ExternalInput")
    ksT = nc.dram_tensor("KsT", (D_IN, L), bf16, kind="ExternalInput")
    vsT = nc.dram_tensor("VsT", (D_IN, L), bf16, kind="ExternalInput")
    wq = nc.dram_tensor("WQ", (D_IN, H * D), bf16, kind="ExternalInput")
    wk = nc.dram_tensor("WK", (D_IN, H * D), bf16, kind="ExternalInput")
    wv = nc.dram_tensor("WV", (D_IN, H * D), bf16, kind="ExternalInput")
    mask = nc.dram_tensor("MASK", (128, 128), bf16, kind="ExternalInput")
    out = nc.dram_tensor("OUT", (H, DA, L), bf16, kind="ExternalOutput")

    with tile.TileContext(nc) as tc:
        build_attention_body(
            tc, qsT[:], ksT[:], vsT[:], wq[:], wk[:], wv[:], mask[:], out[:]
        )
    nc.compile()
    return nc


def get_graph():
    if "nc" not in _GRAPH_CACHE:
        _GRAPH_CACHE["nc"] = _build_graph()
    return _GRAPH_CACHE["nc"]


def make_in_maps(Q_seq, K_seq, V_seq, WQ, WK, WV):
    bf = ml_dtypes.bfloat16
    # fold the softmax 1/sqrt(D) into WQ so no scale is needed on-device
    wq = (np.asarray(WQ, dtype=np.float32) * SCALE).astype(bf)
    wk = np.asarray(WK, dtype=np.float32).astype(bf)
    wv = np.asarray(WV, dtype=np.float32).astype(bf)
    # keep-mask in S^T block coords: row r = key offset, col c = query offset;
    # keep key <= query  <=>  r <= c  (upper triangular incl. diagonal)
    mask = np.triu(np.ones((128, 128), dtype=np.float32)).astype(bf)
    in_maps = []
    for b in range(N_CORES):
        in_maps.append({
            "QsT": np.ascontiguousarray(np.asarray(Q_seq[b], np.float32).T).astype(bf),
            "KsT": np.ascontiguousarray(np.asarray(K_seq[b], np.float32).T).astype(bf),
            "VsT": np.ascontiguousarray(np.asarray(V_seq[b], np.float32).T).astype(bf),
            "WQ": wq,
            "WK": wk,
            "WV": wv,
            "MASK": mask,
        })
    return in_maps


def unshard(results):
    """results: list of per-core {"OUT": [H, DA, L] bf16} -> [B, L, H*D] f32."""
    outs = np.stack(
        [np.asarray(r["OUT"], np.float32) for r in results]
    )                                                    # [B, H, DA, L]
    o = outs[:, :, :D, :] / outs[:, :, D:D + 1, :]       # [B, H, D, L]
    return np.ascontiguousarray(
        o.transpose(0, 3, 1, 2).reshape(B, L, H * D)
    ).astype(np.float32)


def run(inputs, **run_kwargs):
    """Compile + run on the 8 cores; returns (output, BassKernelResults)."""
    from concourse.bass_utils import run_bass_kernel_spmd

    nc = get_graph()
    in_maps = make_in_maps(
        inputs["Q_seq"], inputs["K_seq"], inputs["V_seq"],
        inputs["WQ"], inputs["WK"], inputs["WV"],
    )
    res = run_bass_kernel_spmd(
        nc, in_maps, core_ids=list(range(N_CORES)), **run_kwargs
    )
    return unshard(res.results), res


def kernel(Q_seq, K_seq, V_seq, WQ, WK, WV):
    out, _ = run({
        "Q_seq": Q_seq, "K_seq": K_seq, "V_seq": V_seq,
        "WQ": WQ, "WK": WK, "WV": WV,
    })
    return out



# revision 36
# speedup vs baseline: 1.0528x; 1.0124x over previous
"""Causal multi-head attention (B=8, L=1024, D_IN=512, H=8, D=64) on 8 TRN2
NeuronCores, data-parallel over batch (one batch element per core, no
collectives).

v3 design (per core, batch element b):
  host:   QsT/KsT/VsT = seq[b].T as bf16 [512, 1024]; weights bf16 [512, 512]
          (WQ pre-scaled by 1/sqrt(D) so the softmax scale is free).
  device: qT = WQ.T @ QsT -> [512(dout), 1024(L)] (heads on partitions: head
          2t on partitions 0-63, head 2t+1 on 64-127 of dout tile t); kT
          likewise; v = (VsT.T @ WV) stored [L, H, 66] with ones columns.
          Attention per head-pair t, per query chunk qc (512 cols):
            S^T pair matmuls ROW-TILED (K=64 -> tile_position (0,0)/(64,0))
            so both heads' score matmuls stream concurrently into separate
            PSUM banks of one [128, 2, 512] tile; one exp ACT covers both
            heads; causal mask = 0/1 multiply on the diagonal block; O^T
            accumulated over key tiles with lhsT = [v_h | 1 | 1].
          Pair/chunk groups are streamed in ORDER = [t0qc0, t1qc0, t0qc1,
          t1qc1, t2qc0, ...]: the PE engine queue is FIFO, so the schedule
          keeps DMA-ready work (first-L-half projections of pairs 0-1)
          ahead of work gated on the second sequence-half DMAs.  Projection
          chunks are interleaved between score matmuls (SLOTS) to fill the
          PE while ScalarE exp catches up.
  host:   OUT[h, :64, :] / OUT[h, 64, :] (bf16), transpose, concat heads.

A dependency-free chain of dummy matmuls runs first, overlapping the input
DMAs, so the PE's HAM clock gate opens (1.2 -> 2.4 GHz) before real work.
A tiny dummy exp preloads the ScalarE activation table set off the critical
path.  Each dma_start costs ~0.7-1us of HWDGE descriptor generation on its
issuing engine queue and all transfers share ~436 GB/s of SDMA bandwidth,
so the input staging is a single priority-ordered stream (wq, qsT-half1,
wk, ksT-half1, ...) rather than parallel rings: parallel rings round-robin
the bandwidth and delay the critical first tensors.
"""

import numpy as np
import ml_dtypes

B, L, D_IN = 8, 1024, 512
H, D = 8, 64
DA = D + 2  # head dim + two ones columns (denominator; padded even so the
# bf16 lhsT slices stay 4-byte aligned — odd column counts hang the HW)
N_CORES = 8
SCALE = 1.0 / np.sqrt(D).item()  # folded into WQ on the host
N_WARMUP = 7  # dummy matmuls to open the HAM clock gate during input DMA
N_WARM_MID = (2, 1)  # extra dummy matmuls after q0n0/k0n0 to bridge DMA waits

_GRAPH_CACHE = {}

# score-block groups per query chunk: list of jt lists; jts in one group
# share one spair PSUM tile + one exp ACT (packed along the free dim)
GROUPS_QC = (
    [[0], [1], [2, 3]],
    [[0], [1], [2], [3], [4], [5], [6, 7]],
)


def build_attention_body(tc, qsT, ksT, vsT, wq, wk, wv, mask, out):
    """Emit the per-core kernel into TileContext `tc` (APs per module doc)."""
    import contextlib

    import concourse.mybir as mybir

    nc = tc.nc
    fp32 = mybir.dt.float32
    bf16 = mybir.dt.bfloat16
    EXP = mybir.ActivationFunctionType.Exp

    with contextlib.ExitStack() as ctx:
        const = ctx.enter_context(tc.tile_pool(name="const", bufs=1))
        sb = ctx.enter_context(tc.tile_pool(name="sb", bufs=1))
        ppool = ctx.enter_context(tc.tile_pool(name="ppool", bufs=1))
        stage = ctx.enter_context(tc.tile_pool(name="stage", bufs=2))
        psum = ctx.enter_context(tc.tile_pool(name="psum", bufs=1, space="PSUM"))

        # ---- PE warm-up + ACT table preload, racing the input DMAs -------
        warm_sb = const.tile([128, 512], bf16)
        nc.gpsimd.memset(warm_sb[:], 0.0)
        warm_act = const.tile([128, 16], bf16)
        nc.scalar.activation(warm_act[:], warm_sb[:, 0:16], EXP)
        for i in range(N_WARMUP):
            pwarm = psum.tile(
                [128, 512], fp32, tag="proj", bufs=2, name=f"pwarm_{i}"
            )
            nc.tensor.matmul(
                pwarm[:], warm_sb[:, 0:128], warm_sb[:], start=True, stop=True
            )

        # ---- stage inputs into SBUF (ordered by first use) ---------------
        # One priority-ordered stream on the sync HWDGE ring: q/k first so
        # the score stream starts ASAP, sequence tensors split into L-halves
        # so nch=0 projection chunks start a half-DMA earlier, v last (its
        # projection is deferred into the attention stream).  All transfers
        # share one ~436 GB/s SDMA pool, so parallelizing across rings only
        # delays the critical first tensors; the tiny mask rides the scalar
        # ring since it is off the critical path.
        wq_sb = const.tile([128, 4, 512], bf16)
        nc.sync.dma_start(wq_sb[:], wq.rearrange("(kt p) n -> p kt n", p=128))
        qsT_sb = const.tile([128, 4, L], bf16)
        qsT_r = qsT.rearrange("(kt p) l -> p kt l", p=128)
        nc.sync.dma_start(qsT_sb[:, :, 0:512], qsT_r[:, :, 0:512])
        wk_sb = const.tile([128, 4, 512], bf16)
        nc.sync.dma_start(wk_sb[:], wk.rearrange("(kt p) n -> p kt n", p=128))
        ksT_sb = const.tile([128, 4, L], bf16)
        ksT_r = ksT.rearrange("(kt p) l -> p kt l", p=128)
        nc.sync.dma_start(ksT_sb[:, :, 0:512], ksT_r[:, :, 0:512])
        nc.sync.dma_start(qsT_sb[:, :, 512:L], qsT_r[:, :, 512:L])
        nc.sync.dma_start(ksT_sb[:, :, 512:L], ksT_r[:, :, 512:L])
        wv_sb = const.tile([128, 4, 512], bf16)
        nc.sync.dma_start(wv_sb[:], wv.rearrange("(kt p) n -> p kt n", p=128))
        vsT_sb = const.tile([128, 4, L], bf16)
        vsT_r = vsT.rearrange("(kt p) l -> p kt l", p=128)
        nc.sync.dma_start(vsT_sb[:, :, 0:512], vsT_r[:, :, 0:512])
        nc.sync.dma_start(vsT_sb[:, :, 512:L], vsT_r[:, :, 512:L])
        mask_sb = const.tile([128, 128], bf16)
        nc.scalar.dma_start(mask_sb[:], mask[:, :])

        # ---- persistent activations --------------------------------------
        qT_sb = sb.tile([128, 4, L], bf16)   # [dout%128, dout//128, L]
        kT_sb = sb.tile([128, 4, L], bf16)
        v_sb = sb.tile([128, 8, H, DA], bf16)  # [j%128, j//128, head, d|1|1]
        nc.vector.memset(v_sb[:, :, :, D:DA], 1.0)

        def proj_qk_chunk(t, which, nch):
            # one [128, 512] chunk of qT/kT tile t (lhsT = weight tile)
            dst, w_t, src = (
                (qT_sb, wq_sb, qsT_sb), (kT_sb, wk_sb, ksT_sb)
            )[which]
            pq = psum.tile(
                [128, 512], fp32, tag="proj", bufs=2,
                name=f"pq_{t}_{which}_{nch}",
            )
            for kt in range(4):
                nc.tensor.matmul(
                    pq[:],
                    w_t[:, kt, t * 128:(t + 1) * 128],
                    src[:, kt, nch * 512:(nch + 1) * 512],
                    start=(kt == 0),
                    stop=(kt == 3),
                )
            nc.vector.tensor_copy(
                out=dst[:, t, nch * 512:(nch + 1) * 512], in_=pq[:]
            )

        def proj_v(it):
            # v natural: v[i, n] = sum_k Vs[i, k] WV[k, n]; lhsT = VsT tile
            pv = psum.tile([128, 512], fp32, tag="proj", bufs=2, name=f"pv_{it}")
            for kt in range(4):
                nc.tensor.matmul(
                    pv[:],
                    vsT_sb[:, kt, it * 128:(it + 1) * 128],
                    wv_sb[:, kt, :],
                    start=(kt == 0),
                    stop=(kt == 3),
                )
            nc.vector.tensor_copy(
                out=v_sb[:, it, :, 0:D],
                in_=pv.rearrange("p (h d) -> p h d", h=H),
            )

        # ---- flat attention stream across all pairs ----------------------
        # Groups G0..G39 ordered so early groups only need the FIRST L-half
        # projections (t0qc0, t1qc0) while qsT2/ksT2 are still in flight;
        # the nch=1-gated groups (t0qc1) come after.  The PE engine queue is
        # FIFO, so a data-gated matmul at the queue head stalls everything
        # behind it — this order keeps DMA-ready work ahead of gated work.
        ORDER = [(0, 0), (1, 0), (0, 1), (1, 1), (2, 0), (2, 1), (3, 0), (3, 1)]
        flat = []  # (t, qc, group jts, last_of_qc)
        for t, qc in ORDER:
            gs = GROUPS_QC[qc]
            for gi, g in enumerate(gs):
                flat.append((t, qc, g, gi == len(gs) - 1))

        # slot proj chunks, one per step (smooth PE/Scalar interleave),
        # placed after their input DMA lands but before their consumer
        # group's scores: nch=1 chunks need the second sequence L-halves
        # (~8us), v chunks need vsT (~11-13us).
        SLOTS = {
            0: [("qk", 1, 0, 0)], 1: [("qk", 1, 1, 0)],
            3: [("qk", 0, 0, 1)], 4: [("qk", 0, 1, 1)],
            6: [("qk", 1, 0, 1)], 7: [("qk", 1, 1, 1)],
            8: [("qk", 2, 0, 0)], 10: [("qk", 2, 1, 0)],
            9: [("v", 0)], 11: [("v", 1)], 12: [("v", 2)], 13: [("v", 3)],
            14: [("v", 4)], 15: [("v", 5)], 16: [("v", 6)], 17: [("v", 7)],
            18: [("qk", 2, 0, 1)], 20: [("qk", 2, 1, 1)],
            22: [("qk", 3, 0, 0)], 24: [("qk", 3, 1, 0)],
            27: [("qk", 3, 0, 1)], 29: [("qk", 3, 1, 1)],
        }
        CAP1_UNTIL = 17  # O-drain cap is 1/step through the v window

        oT_d = {}  # (t, qc) -> oT psum tile, allocated at first O drain

        def drain_o(rec):
            t, qc, group, last_of_qc, colofs_l, pexp = rec
            qlo = 512 * qc
            nj = 4 * (qc + 1)
            if (t, qc) not in oT_d:
                oT_d[(t, qc)] = psum.tile(
                    [DA, 2, 512], fp32, tag="oT", bufs=1, name=f"oT_{t}_{qc}"
                )
            oT = oT_d[(t, qc)]
            for jt, cofs in zip(group, colofs_l):
                j0 = 128 * jt
                off = max(0, j0 - qlo)
                cw = 512 - off
                for hh in range(2):
                    nc.tensor.matmul(
                        oT[:, hh, off:off + cw],
                        v_sb[:, jt, 2 * t + hh, :],
                        pexp[:, hh, cofs:cofs + cw],
                        start=(jt == 0),
                        stop=(jt == nj - 1),
                        skip_group_check=True,
                    )
            if (t, qc) == (3, 1) and 5 in group:
                # final qc: cols [0:256] are final after jt5 — ship them while
                # the last two groups still run, so the kernel tail is short
                o_sta = stage.tile(
                    [DA, 2, 512], bf16, tag="ost", name="ost_3_1a"
                )
                nc.vector.tensor_copy(out=o_sta[:, :, 0:256], in_=oT[:, :, 0:256])
                # both heads on the sync ring: a descriptor on the scalar
                # queue would steal ~0.8us from the exp stream, which paces
                # this region
                for hh in range(2):
                    nc.sync.dma_start(
                        out[2 * t + hh, :, qlo:qlo + 256], o_sta[:, hh, 0:256]
                    )
            if last_of_qc:
                o_st = stage.tile(
                    [DA, 2, 512], bf16, tag="ost", name=f"ost_{t}_{qc}"
                )
                lo = 256 if (t, qc) == (3, 1) else 0
                if lo == 0:
                    nc.vector.tensor_copy(out=o_st[:, :, 0:384],
                                          in_=oT[:, :, 0:384])
                    nc.vector.tensor_copy(out=o_st[:, :, 384:512],
                                          in_=oT[:, :, 384:512])
                else:
                    nc.vector.tensor_copy(out=o_st[:, :, 256:512],
                                          in_=oT[:, :, 256:512])
                # mid-kernel pieces keep the scalar queue clear for the exp
                # stream (descriptor generation costs ~0.8us on its queue);
                # only the kernel-final piece splits across both rings, when
                # no exp remains to displace
                if (t, qc) == (3, 1):
                    engs = (nc.sync, nc.scalar)
                else:
                    engs = (nc.sync, nc.sync)
                for hh, eng in zip(range(2), engs):
                    eng.dma_start(
                        out[2 * t + hh, :, qlo + lo:qlo + 512],
                        o_st[:, hh, lo:512],
                    )

        # lead-in: first projection chunks for pair 0 (first L-half only),
        # with dummy-matmul filler so the PE never idles past a HAM window
        # while the k/q DMAs land
        def warm_fill(n, base):
            for i in range(n):
                pw = psum.tile(
                    [128, 512], fp32, tag="proj", bufs=2,
                    name=f"pwm_{base}_{i}",
                )
                nc.tensor.matmul(
                    pw[:], warm_sb[:, 0:128], warm_sb[:],
                    start=True, stop=True,
                )

        proj_qk_chunk(0, 0, 0)
        warm_fill(N_WARM_MID[0], "a")
        proj_qk_chunk(0, 1, 0)
        warm_fill(N_WARM_MID[1], "b")

        pending = []   # group records awaiting O emission
        n_v = 0        # v chunks emitted so far (in it order 0..7)

        def emit_group(Gi):
            # S-pair matmuls -> exp ACT -> diag masks for flat group Gi
            t, qc, group, last_of_qc = flat[Gi]
            qlo = 512 * qc
            sp = psum.tile(
                [128, 2, 512], fp32, tag="spair", bufs=2, name=f"sp_{Gi}"
            )
            pexp = ppool.tile(
                [128, 2, 512], bf16, tag="P", bufs=14, name=f"P_{Gi}"
            )
            # first groups: high priority so the scheduler doesn't stuff
            # hoisted projection filler between the k0n0 cast and S(G0)
            ctx2 = None
            if Gi < 7:
                ctx2 = tc.high_priority(offset=50000)
                ctx2.__enter__()
            colofs_l = []
            colofs = 0
            for jt in group:
                j0 = 128 * jt
                off = max(0, j0 - qlo)
                cw = 512 - off
                for hh in range(2):
                    pb = 64 * hh
                    nc.tensor.matmul(
                        sp[:, hh, colofs:colofs + cw],
                        kT_sb[pb:pb + 64, t, j0:j0 + 128],
                        qT_sb[pb:pb + 64, t, qlo + off:qlo + 512],
                        start=True,
                        stop=True,
                    )
                colofs_l.append(colofs)
                colofs += cw
            nc.scalar.activation(pexp[:, :, 0:colofs], sp[:, :, 0:colofs], EXP)
            for jt, cofs in zip(group, colofs_l):
                if 128 * jt >= qlo:  # chunk starts at the diagonal
                    for hh in range(2):
                        nc.vector.tensor_mul(
                            pexp[:, hh, cofs:cofs + 128],
                            pexp[:, hh, cofs:cofs + 128],
                            mask_sb[:],
                        )
            if ctx2 is not None:
                ctx2.__exit__(None, None, None)
            pending.append((t, qc, group, last_of_qc, colofs_l, pexp))

        # Groups are emitted in PAIRS: S(G) and S(G+1) go back-to-back into
        # the two spair buffers, so ScalarE always has a 2-deep exp input
        # queue and rides out the projection/drain fill phases without
        # starving (exp consumes a group ~3x slower than the PE produces
        # one, but fill bursts between single S-pairs exceeded one exp).
        for base in range(0, len(flat), 2):
            emit_group(base)
            emit_group(base + 1)
            for Gi in (base, base + 1):
                for c in SLOTS.get(Gi, []):
                    if c[0] == "qk":
                        proj_qk_chunk(c[1], c[2], c[3])
                    else:
                        proj_v(c[1])
                        n_v += 1
            # drain eligible pending O groups: light through the v window
            # (PE-dense region), then work the backlog off
            cap = 2 if base + 1 <= CAP1_UNTIL else (3 if base + 1 <= 33 else 4)
            drained = 0
            while pending and drained < cap:
                need_v = max(pending[0][2])
                if len(pending) > 1 and need_v < n_v:
                    drain_o(pending.pop(0))
                    drained += 1
                else:
                    break
        while pending:
            drain_o(pending.pop(0))


def _build_graph():
    import concourse.mybir as mybir
    import concourse.tile as tile
    from concourse import bacc

    nc = bacc.Bacc("TRN2", target_bir_lowering=False)
    bf16 = mybir.dt.bfloat16
    qsT = nc.dram_tensor("QsT", (D_IN, L), bf16, kind="ExternalInput")
    ksT = nc.dram_tensor("KsT", (D_IN, L), bf16, kind="ExternalInput")
    vsT = nc.dram_tensor("VsT", (D_IN, L), bf16, kind="ExternalInput")
    wq = nc.dram_tensor("WQ", (D_IN, H * D), bf16, kind="ExternalInput")
    wk = nc.dram_tensor("WK", (D_IN, H * D), bf16, kind="ExternalInput")
    wv = nc.dram_tensor("WV", (D_IN, H * D), bf16, kind="ExternalInput")
    mask = nc.dram_tensor("MASK", (128, 128), bf16, kind="ExternalInput")
    out = nc.dram_tensor("OUT", (H, DA, L), bf16, kind="ExternalOutput")

    with tile.TileContext(nc) as tc:
        build_attention_body(
            tc, qsT[:], ksT[:], vsT[:], wq[:], wk[:], wv[:], mask[:], out[:]
        )
    nc.compile()
    return nc


def get_graph():
    if "nc" not in _GRAPH_CACHE:
        _GRAPH_CACHE["nc"] = _build_graph()
    return _GRAPH_CACHE["nc"]


def make_in_maps(Q_seq, K_seq, V_seq, WQ, WK, WV):
    bf = ml_dtypes.bfloat16
    # fold the softmax 1/sqrt(D) into WQ so no scale is needed on-device
    wq = (np.asarray(WQ, dtype=np.float32) * SCALE).astype(bf)
    wk = np.asarray(WK, dtype=np.float32).astype(bf)
    wv = np.asarray(WV, dtype=np.float32).astype(bf)
    # keep-mask in S^T block coords: row r = key offset, col c = query offset;
    # keep key <= query  <=>  r <= c  (upper triangular incl. diagonal)
    mask = np.triu(np.ones((128, 128), dtype=np.float32)).astype(bf)
    in_maps = []
    for b in range(N_CORES):
        in_maps.append({
            "QsT": np.ascontiguousarray(np.asarray(Q_seq[b], np.float32).T).astype(bf),
            "KsT": np.ascontiguousarray(np.asarray(K_seq[b], np.float32).T).astype(bf),
            "VsT": np.ascontiguousarray(np.asarray(V_seq[b], np.float32).T).astype(bf),
            "WQ": wq,
            "WK": wk,
            "WV": wv,
            "MASK": mask,
        })
    return in_maps


def unshard(results):
    """results: list of per-core {"OUT": [H, DA, L] bf16} -> [B, L, H*D] f32."""
    outs = np.stack(
        [np.asarray(r["OUT"], np.float32) for r in results]
    )                                                    # [B, H, DA, L]
    o = outs[:, :, :D, :] / outs[:, :, D:D + 1, :]       # [B, H, D, L]
    return np.ascontiguousarray(
        o.transpose(0, 3, 1, 2).reshape(B, L, H * D)
    ).astype(np.float32)


def run(inputs, **run_kwargs):
    """Compile + run on the 8 cores; returns (output, BassKernelResults)."""
    from concourse.bass_utils import run_bass_kernel_spmd

    nc = get_graph()
    in_maps = make_in_maps(
        inputs["Q_seq"], inputs["K_seq"], inputs["V_seq"],
        inputs["WQ"], inputs["WK"], inputs["WV"],
    )
    res = run_bass_kernel_spmd(
        nc, in_maps, core_ids=list(range(N_CORES)), **run_kwargs
    )
    return unshard(res.results), res


def kernel(Q_seq, K_seq, V_seq, WQ, WK, WV):
    out, _ = run({
        "Q_seq": Q_seq, "K_seq": K_seq, "V_seq": V_seq,
        "WQ": WQ, "WK": WK, "WV": WV,
    })
    return out

